# revision 37
# baseline (speedup 1.0000x reference)
"""Trainium2 Bass kernel for nn_MemoryReader (sparse_attention).

Reference computation (per batch b):
  s[m,q]  = sum_c K_M[b,c,m] * K_Q[b,c,q] / sqrt(64)        m in [0,9216), q in [0,2304)
  attn    = softmax over m
  mem[c,q]= sum_m V_M[b,c,m] * attn[m,q]                    c in [0,128)
  E       = concat([mem, V_Q[b]], ch)                       [256, q]
  out     = relu(bn_scale * (conv_w @ E) + bn_shift)        [64, q]

Sharding: 8 cores = (B=4) x (Q halves of 1152). Fully data-parallel, no
collectives. Within a core, Q is processed in 2 passes of 576 columns
(PSUM capacity), softmax over the full M=9216 without max-subtraction
(scores are ~N(0,1): exp is safe in fp32/bf16 range).

Per-core inputs (host-prepared), packed to minimize DMA semaphore domains
(walrus allows a single sync-wait per compute instruction and a limited
wait list on the kernel-tail drain):
  kmq [128, 5824] bf16 : cols 0:4608   km_packed (even m-tiles on
                         partitions 0-63, odd on 64-127 -> QK matmul pairs
                         run concurrently in PE row groups, contract=64)
                         cols 4608:5760 kq_dup (K_Q slice, pre-scaled by
                         1/sqrt(64), duplicated on partitions 64-127)
                         cols 5760:5824 w1t (BN-folded conv W for the mem
                         half, transposed)
  vt  [128, 9216] bf16 : V_M transposed per 128-tile: vt[p, t*128+c] =
                         V_M[c, t*128+p]  (PV lhsT, contract on m)
  vqw [128, 1217] f32  : cols 0:1152 V_Q slice, cols 1152:1216 w2t,
                         col 1216 (rows 0:64) BN shift
Output: out [64, 1152] f32.
"""

import numpy as np
import ml_dtypes

import concourse.bass as bass
from concourse import bacc
import concourse.mybir as mybir
import concourse.tile as tile
from concourse.tile_rust import add_dep_helper

B, C_K, C_V, NN, H, W = 4, 64, 128, 4, 48, 48
M = NN * H * W          # 9216
Q = H * W               # 2304
QH = Q // 2             # 1152 per core
QP = QH // 2            # 576 per in-kernel pass
OUT_CH = 64
BN_EPS = 1e-5
NCORES = 8
MT = M // 128           # 72 m-tiles
KMW = (MT // 2) * 128   # 4608
BF16 = mybir.dt.bfloat16
F32 = mybir.dt.float32
AF = mybir.ActivationFunctionType


def _emit(nc, aps):
    kmq, vt, vqw, out = aps
    with tile.TileContext(nc) as tc:
        with (
            tc.tile_pool(name="consts", bufs=1) as consts,
            tc.tile_pool(name="pp", bufs=8) as pp,
            tc.tile_pool(name="epi", bufs=2) as epi,
            tc.tile_pool(name="obuf", bufs=1) as obuf,
            tc.tile_pool(name="dacc", bufs=2) as daccp,
            tc.tile_pool(name="spool", bufs=3, space="PSUM") as spool,
            tc.tile_pool(name="mpool", bufs=1, space="PSUM") as mpool,
        ):
            kmq_t = consts.tile([128, KMW + QH + OUT_CH], BF16)
            vt_t = consts.tile([128, M], BF16)
            vqw_t = consts.tile([128, QH + OUT_CH + 1], F32)
            ones_t = consts.tile([128, OUT_CH], BF16)

            # chunked loads: early m-tiles unblock before the tails arrive
            nc.sync.dma_start(out=kmq_t[:, KMW:KMW + QP], in_=kmq[:, KMW:KMW + QP])
            nc.sync.dma_start(out=kmq_t[:, 0:256], in_=kmq[:, 0:256])
            nc.sync.dma_start(out=kmq_t[:, 256:1152], in_=kmq[:, 256:1152])
            nc.sync.dma_start(out=kmq_t[:, KMW + QP:], in_=kmq[:, KMW + QP:])
            nc.sync.dma_start(out=vt_t[:, 0:2304], in_=vt[:, 0:2304])
            nc.sync.dma_start(out=vqw_t, in_=vqw)
            nc.sync.dma_start(out=kmq_t[:, 1152:KMW], in_=kmq[:, 1152:KMW])
            for i in range(1, 4):
                nc.sync.dma_start(out=vt_t[:, i * 2304:(i + 1) * 2304],
                                  in_=vt[:, i * 2304:(i + 1) * 2304])
            # DVE rewrites vqw and then produces ones: a single DVE
            # semaphore covers both; PE observes it via one dummy ldweights
            vqw_cp = nc.vector.tensor_copy(out=vqw_t, in_=vqw_t)
            ones_ms = nc.vector.memset(ones_t, 1.0)
            # order the memset after the copy on the Pool queue so one
            # ldweights absorber (Pool >= memset) covers the vqw rewrite too
            add_dep_helper(ones_ms.ins, vqw_cp.ins, sync=False,
                           reason="ones memset after vqw copy")

            kq0 = KMW                     # kq columns inside kmq_t
            w1c = KMW + QH                # w1t columns inside kmq_t
            vq_t = vqw_t[:, 0:QH]
            w2t_t = vqw_t[:, QH:QH + OUT_CH]
            shift_t = vqw_t[0:OUT_CH, QH + OUT_CH:QH + OUT_CH + 1]

            o_t = obuf.tile([OUT_CH, QH], F32, tag="o")

            LAG = 5
            p_hist = []
            for rep in range(reps):
              for p in range(2):
                  qs = p * QP
                  # [128, 1024] = 2 PSUM banks, one accumulation group per
                  # bank: mem in cols 0:512 (bank 0) + 512:576 (bank 1).
                  # The softmax denominator accumulates on DVE (pair-tree).
                  mem_t = mpool.tile([128, 1024], F32, tag="mem")
                  g_t = daccp.tile([128, QP], BF16, tag="g")
                  p_by_mt = {}
                  # software pipeline: emit QK(mt)/exp(mt) LAG steps ahead of
                  # PV(mt-LAG) so the in-order PE queue never stalls on exp
                  for mt in range(MT + LAG):
                    if mt < MT:
                      bp = 64 * (mt % 2)
                      cl = (mt // 2) * 128
                      lhs = kmq_t[bp:bp + 64, cl:cl + 128]
                      if len(p_hist) >= 3:
                          # absorbs the s-slot WAR (the exp lagging exactly a
                          # full spool rotation) into a 1-wait ldweights; the
                          # QK below then only carries its PSUM WAW wait
                          nc.tensor.ldweights(p_hist[-3][0:1, 0:2])
                      s_t = spool.tile([128, 1024], F32, tag="s")
                      qk0 = nc.tensor.matmul(
                          s_t[:, 0:512], lhs,
                          kmq_t[bp:bp + 64, kq0 + qs:kq0 + qs + 512],
                          start=True, stop=True)
                      qk1 = nc.tensor.matmul(
                          s_t[:, 512:576], lhs,
                          kmq_t[bp:bp + 64, kq0 + qs + 512:kq0 + qs + QP],
                          start=True, stop=True)
                      if p == 0 and mt == 0 and rep == 0:
                          # absorbers: PE observes the vt DMA and the DVE
                          # (vqw copy + ones memset) with one wait each
                          nc.tensor.ldweights(vt_t[:, 0:1])
                          nc.tensor.ldweights(ones_t[:, 0:1])
                      p_t = pp.tile([128, QP], BF16, tag="p")
                      nc.scalar.activation(out=p_t, in_=s_t[:, 0:QP], func=AF.Exp)
                      p_by_mt[mt] = p_t
                      p_hist.append(p_t)
                      if len(p_hist) > 6:
                          p_hist.pop(0)
                      # d pair-tree on DVE: e = p(even)+p(odd) waits only on
                      # ACT; the g fold waits only on DVE
                      if mt % 2 == 1:
                          if mt == 1:
                              nc.vector.tensor_add(g_t, p_hist[-2], p_t)
                          else:
                              e_t = pp.tile([128, QP], BF16, tag="e")
                              nc.vector.tensor_add(e_t, p_hist[-2], p_t)
                              nc.vector.tensor_add(g_t, g_t, e_t)
                    if mt >= LAG:
                      pv = mt - LAG
                      pvp = p_by_mt.pop(pv)
                      vl = vt_t[:, pv * 128:(pv + 1) * 128]
                      st, sp = (pv == 0), (pv == MT - 1)
                      nc.tensor.matmul(
                          mem_t[:, 0:512], vl, pvp[:, 0:512], start=st, stop=sp)
                      nc.tensor.matmul(
                          mem_t[:, 512:QP], vl, pvp[:, 512:QP], start=st, stop=sp)

                  # epilogue for this pass; the y1/y2/d_red matmuls reuse the
                  # now-free spool / mem_t PSUM banks
                  mem_sb = epi.tile([128, QP], BF16, tag="mem_sb")
                  nc.vector.tensor_copy(out=mem_sb, in_=mem_t[:, 0:QP])
                  # d_red and y1 reuse the two freed mem_t banks (d on
                  # partitions 64:128, y1 on 0:64) so the spool slots stay
                  # available for the next pass's QKs
                  nc.tensor.matmul(mem_t[64:128, 0:512], ones_t[:, 0:64],
                                   g_t[:, 0:512], start=True, stop=True)
                  nc.tensor.matmul(mem_t[64:128, 512:QP], ones_t[:, 0:64],
                                   g_t[:, 512:QP], start=True, stop=True)
                  r_t = epi.tile([64, QP], F32, tag="r")
                  nc.vector.reciprocal(out=r_t, in_=mem_t[64:128, 0:QP])

                  y2 = spool.tile([64, 1024], F32, tag="s")
                  nc.tensor.matmul(y2[0:64, 0:512], w2t_t[:, 0:64],
                                   vq_t[:, qs:qs + 512], start=True, stop=True)
                  nc.tensor.matmul(y2[0:64, 512:QP], w2t_t[:, 0:64],
                                   vq_t[:, qs + 512:qs + QP], start=True, stop=True)
                  nc.tensor.ldweights(mem_sb[:, 0:1])
                  nc.tensor.matmul(mem_t[0:64, 0:512], kmq_t[:, w1c:w1c + OUT_CH],
                                   mem_sb[:, 0:512], start=True, stop=True)
                  nc.tensor.matmul(mem_t[0:64, 512:QP], kmq_t[:, w1c:w1c + OUT_CH],
                                   mem_sb[:, 512:QP], start=True, stop=True)

                  # DVE observes its own r_t value so the u-mul needs only the
                  # PE wait (y1, which also covers y2)
                  rabs = epi.tile([64, 2], F32, tag="rabs")
                  nc.vector.tensor_copy(out=rabs, in_=r_t[:, 0:2])
                  u_t = epi.tile([64, QP], F32, tag="u")
                  nc.vector.tensor_mul(u_t, mem_t[0:64, 0:QP], r_t)
                  v_t = epi.tile([64, QP], F32, tag="v")
                  nc.vector.tensor_add(v_t, u_t, y2[0:64, 0:QP])
                  nc.vector.tensor_scalar(
                      out=o_t[:, qs:qs + QP], in0=v_t, scalar1=shift_t,
                      scalar2=0.0, op0=mybir.AluOpType.add,
                      op1=mybir.AluOpType.max)
            nc.sync.dma_start(out=out, in_=o_t)


def _build_nc():
    nc = bacc.Bacc("TRN2", target_bir_lowering=False, debug=False)
    kmq = nc.dram_tensor("kmq", [128, KMW + QH + OUT_CH], BF16,
                         kind="ExternalInput").ap()
    vt = nc.dram_tensor("vt", [128, M], BF16, kind="ExternalInput").ap()
    vqw = nc.dram_tensor("vqw", [128, QH + OUT_CH + 1], F32,
                         kind="ExternalInput").ap()
    out = nc.dram_tensor("out", [OUT_CH, QH], F32, kind="ExternalOutput").ap()
    _emit(nc, (kmq, vt, vqw, out))
    nc.compile()
    return nc


def prepare_in_maps(K_M, V_M, K_Q, V_Q, conv_w, bn_gamma, bn_beta, bn_mean, bn_var):
    """Host-side shard + layout prep. Returns list of 8 per-core input dicts."""
    bf16 = ml_dtypes.bfloat16
    K_M = np.asarray(K_M, np.float32)
    V_M = np.asarray(V_M, np.float32)
    K_Q = np.asarray(K_Q, np.float32)
    V_Q = np.asarray(V_Q, np.float32)
    conv_w = np.asarray(conv_w, np.float32)
    scale = np.asarray(bn_gamma, np.float32) / np.sqrt(
        np.asarray(bn_var, np.float32) + BN_EPS)
    shift = (np.asarray(bn_beta, np.float32)
             - np.asarray(bn_mean, np.float32) * scale)
    w_eff = conv_w * scale[:, None]
    w1t = np.ascontiguousarray(w_eff[:, :C_V].T)          # [128, 64]
    w2t = np.ascontiguousarray(w_eff[:, C_V:].T)          # [128, 64]

    in_maps = []
    for b in range(B):
        km_full = K_M[b].reshape(C_K, M)                  # [64, 9216]
        km_r = km_full.reshape(C_K, MT, 128)
        km_packed = np.empty((128, KMW), np.float32)
        km_packed[0:64] = km_r[:, 0::2, :].reshape(C_K, -1)
        km_packed[64:128] = km_r[:, 1::2, :].reshape(C_K, -1)

        v_full = V_M[b].reshape(C_V, M)
        vt = np.ascontiguousarray(
            v_full.reshape(C_V, MT, 128).transpose(2, 1, 0).reshape(128, M)
        ).astype(bf16)

        kq_full = K_Q[b].reshape(C_K, Q) * (1.0 / np.sqrt(C_K))
        vq_full = V_Q[b].reshape(C_V, Q)
        for h in range(2):
            sl = slice(h * QH, (h + 1) * QH)
            kq_half = kq_full[:, sl]
            kmq = np.empty((128, KMW + QH + OUT_CH), np.float32)
            kmq[:, 0:KMW] = km_packed
            kmq[0:64, KMW:KMW + QH] = kq_half
            kmq[64:128, KMW:KMW + QH] = kq_half
            kmq[:, KMW + QH:] = w1t
            vqw = np.zeros((128, QH + OUT_CH + 1), np.float32)
            vqw[:, 0:QH] = vq_full[:, sl]
            vqw[:, QH:QH + OUT_CH] = w2t
            vqw[0:OUT_CH, QH + OUT_CH] = shift
            in_maps.append({
                "kmq": kmq.astype(bf16),
                "vt": vt,
                "vqw": vqw,
            })
    return in_maps


def assemble_output(results):
    """results: list of 8 dicts with 'out' [64, 1152] -> [4, 64, 48, 48] f32."""
    out = np.empty((B, OUT_CH, Q), np.float32)
    for c in range(NCORES):
        b, h = c // 2, c % 2
        out[b, :, h * QH:(h + 1) * QH] = results[c]["out"]
    return out.reshape(B, OUT_CH, H, W)


_RUNNER = None


def _get_runner():
    """Build the Bass module + a cached sharded jit callable (compile once)."""
    global _RUNNER
    if _RUNNER is not None:
        return _RUNNER
    import jax
    from jax.sharding import Mesh, PartitionSpec
    from jax.experimental.shard_map import shard_map
    from concourse import bass2jax

    nc = _build_nc()
    bass2jax.install_neuronx_cc_hook()

    partition_name = nc.partition_id_tensor.name if nc.partition_id_tensor else None
    in_names, out_names, out_avals, zero_outs = [], [], [], []
    for alloc in nc.m.functions[0].allocations:
        if not isinstance(alloc, mybir.MemoryLocationSet):
            continue
        name = alloc.memorylocations[0].name
        if alloc.kind == "ExternalInput":
            if name != partition_name:
                in_names.append(name)
        elif alloc.kind == "ExternalOutput":
            out_names.append(name)
            shape = tuple(alloc.tensor_shape)
            dtype = mybir.dt.np(alloc.dtype)
            out_avals.append(jax.core.ShapedArray(shape, dtype))
            zero_outs.append(np.zeros(shape, dtype))
    n_params = len(in_names)
    n_outs = len(out_avals)
    all_in_names = in_names + out_names
    if partition_name is not None:
        all_in_names = all_in_names + [partition_name]

    def _body(*args):
        operands = list(args)
        if partition_name is not None:
            operands.append(bass2jax.partition_id_tensor())
        outs = bass2jax._bass_exec_p.bind(
            *operands,
            out_avals=tuple(out_avals),
            in_names=tuple(all_in_names),
            out_names=tuple(out_names),
            lowering_input_output_aliases=(),
            sim_require_finite=True,
            sim_require_nnan=True,
            nc=nc,
        )
        return tuple(outs)

    devices = jax.devices()[:NCORES]
    assert len(devices) == NCORES, f"need {NCORES} devices, got {len(jax.devices())}"
    mesh = Mesh(np.asarray(devices), ("core",))
    in_specs = (PartitionSpec("core"),) * (n_params + n_outs)
    out_specs = (PartitionSpec("core"),) * n_outs
    donate = tuple(range(n_params, n_params + n_outs))
    sharded = jax.jit(
        shard_map(_body, mesh=mesh, in_specs=in_specs, out_specs=out_specs,
                  check_rep=False),
        donate_argnums=donate, keep_unused=True,
    )
    _RUNNER = (sharded, in_names, out_names, out_avals, zero_outs)
    return _RUNNER


def run_cores(in_maps):
    """Run the 8-core SPMD program; returns per-core output dicts."""
    sharded, in_names, out_names, out_avals, zero_outs = _get_runner()
    concat_in = [
        np.concatenate([np.asarray(in_maps[c][n]) for c in range(NCORES)], axis=0)
        for n in in_names
    ]
    concat_zeros = [
        np.zeros((NCORES * z.shape[0], *z.shape[1:]), z.dtype) for z in zero_outs
    ]
    out_arrs = sharded(*concat_in, *concat_zeros)
    return [
        {
            name: np.asarray(out_arrs[i]).reshape(NCORES, *out_avals[i].shape)[c]
            for i, name in enumerate(out_names)
        }
        for c in range(NCORES)
    ]


def kernel(K_M, V_M, K_Q, V_Q, conv_w, bn_gamma, bn_beta, bn_mean, bn_var):
    in_maps = prepare_in_maps(K_M, V_M, K_Q, V_Q, conv_w,
                              bn_gamma, bn_beta, bn_mean, bn_var)
    results = run_cores(in_maps)
    return assemble_output(results)
